# revision 8
# baseline (speedup 1.0000x reference)
"""Trainium2 Bass kernel for nn_EqAMPBC (FWM/XPM nonlinear equalizer).

Strategy: pure data-parallel over 8 NeuronCores (batch 131072 -> 16384/core).
Per core, samples are processed in 32 chunks of N=512 in a transposed layout
(features on partitions, samples on the free dim):
  - one-hot fp32r matmuls on TensorE gather the FWM triplet operand rows,
  - VectorE forms the 4 real product tensors per (h, mode),
  - TensorE contracts products -> As -> t (the W-weighted FWM sums),
  - a final TensorE reduction builds 12 per-sample scalars (FWM sums, z,
    phase pre-sums, center taps), which are PE-transposed into a
    sample-major megatile where ScalarE/VectorE apply exp/sin/cos and the
    final complex combine.
All engine work happens on device; the host only reshapes/shards.
"""
import sys
import numpy as np

sys.path.insert(0, "/opt/trn_rl_repo")

M = 41
P = 20
RHO = 1.0
NCORES = 8
N = 512
F_ROWS = 8


def _fwm_index():
    h = M // 2
    ms, ns = [], []
    for m in range(-h, h + 1):
        for n in range(-h, h + 1):
            if m != 0 and n != 0 and abs(m * n) <= RHO * h and abs(m + n) <= h:
                ms.append(m)
                ns.append(n)
    return np.asarray(ms, np.int32), np.asarray(ns, np.int32)


M_IDX, N_IDX = _fwm_index()
HDIM = len(M_IDX)
A_TAP = P + N_IDX
C_TAP = P + M_IDX + N_IDX
D_TAP = P + M_IDX
NPROD = 2 * HDIM
CHUNKS = [(0, 128), (128, 128), (256, 128), (384, 128), (512, 8)]


def _build_tables(fwm_wr, fwm_wi, conv1_w, conv2_w, C00):
    t = {}
    r_all = np.arange(NPROD)
    h_all, j_all = r_all // 2, r_all % 2
    src_a = j_all * 41 + A_TAP[h_all]
    src_c = j_all * 41 + C_TAP[h_all]
    for side, src in (("a", src_a), ("c", src_c)):
        for u, (o, ln) in enumerate(CHUNKS):
            g = np.zeros((82, ln), np.float32)
            g[src[o:o + ln], np.arange(ln)] = 1.0
            t[f"ga_{side}_{u}"] = g
    for u, (o, ln) in enumerate(CHUNKS):
        w_r = np.zeros((ln, ln), np.float32)
        w_i3 = np.zeros((ln, ln), np.float32)
        w_i4 = np.zeros((ln, ln), np.float32)
        loc = np.arange(ln)
        w_r[loc, 2 * (loc // 2)] = 1.0
        w_i3[loc, 2 * (loc // 2) + 1] = 1.0
        w_i4[loc, 2 * (loc // 2) + 1] = -1.0
        t[f"w1_T12_{u}"] = w_r
        t[f"w1_T3_{u}"] = w_i3
        t[f"w1_T4_{u}"] = w_i4
    for v, (o, ln) in enumerate(CHUNKS):
        wr = np.zeros((ln, 82), np.float32)
        wi = np.zeros((ln, 82), np.float32)
        for rl in range(ln):
            h = (o + rl) // 2
            part = rl % 2
            for i in range(2):
                col = i * 41 + D_TAP[h]
                if part == 0:
                    wr[rl, col] += fwm_wr[i, h]
                    wi[rl, col] += fwm_wi[i, h]
                else:
                    wr[rl, col] += -fwm_wi[i, h]
                    wi[rl, col] += fwm_wr[i, h]
        t[f"w2r_{v}"] = wr
        t[f"w2i_{v}"] = wi
    w1z = conv1_w.copy(); w1z[P] = 0.0
    w2z = conv2_w.copy(); w2z[P] = 0.0
    q1 = np.zeros((82, F_ROWS), np.float32)
    q2 = np.zeros((82, F_ROWS), np.float32)
    q3 = np.zeros((82, F_ROWS), np.float32)
    q4 = np.zeros((82, F_ROWS), np.float32)
    for i in range(2):
        rows = np.arange(41) + i * 41
        q1[rows, 2 * i + 0] = 0.5
        q2[rows, 2 * i + 0] = -0.5
        q3[rows, 2 * i + 1] = 0.5
        q4[rows, 2 * i + 1] = 0.5
    t["r3_q1"], t["r3_q2"], t["r3_q3"], t["r3_q4"] = q1, q2, q3, q4
    pw = np.zeros((82, F_ROWS), np.float32)
    for i in range(2):
        for tap in range(41):
            r = i * 41 + tap
            pw[r, 6] += (2.0 if i == 0 else 1.0) * w1z[tap]
            pw[r, 7] += (2.0 if i == 1 else 1.0) * w1z[tap]
        pw[i * 41 + P, 6] += 0.5 * C00
        pw[i * 41 + P, 7] += 0.5 * C00
    t["r3_pw"] = pw
    xrA = np.zeros((128, F_ROWS), np.float32)
    xrA[np.arange(41), 4] = 0.5 * w2z
    xrA[np.arange(41) + 64, 4] = 0.5 * w2z
    xrB = np.zeros((128, F_ROWS), np.float32)
    xrB[np.arange(41), 5] = 0.5 * w2z
    xrB[np.arange(41) + 64, 5] = -0.5 * w2z
    t["r3_xrA"], t["r3_xrB"] = xrA, xrB
    t["ident8"] = np.eye(F_ROWS, dtype=np.float32)
    return t


_CACHED = {}


def _build_program(Bc):
    import concourse.bacc as bacc
    import concourse.mybir as mybir
    import concourse.tile as tile

    f32 = mybir.dt.float32
    bf16 = mybir.dt.bfloat16
    Act = mybir.ActivationFunctionType
    Op = mybir.AluOpType
    NCHUNK = Bc // N
    MCOLS = Bc // 128          # megatile cols per quantity-slot group

    nc = bacc.Bacc("TRN2", target_bir_lowering=False, debug=False,
                   num_devices=NCORES)

    dXR = nc.dram_tensor("XR", [82, Bc], bf16, kind="ExternalInput").ap()
    dXI = nc.dram_tensor("XI", [82, Bc], bf16, kind="ExternalInput").ap()
    dXC = nc.dram_tensor("XC", [128, 4 * MCOLS], f32, kind="ExternalInput").ap()
    dT0 = nc.dram_tensor("T0M", [128, MCOLS], f32, kind="ExternalInput").ap()
    tab_shapes = {}
    tabs0 = _build_tables(np.zeros((2, HDIM), np.float32),
                          np.zeros((2, HDIM), np.float32),
                          np.zeros(M, np.float32), np.zeros(M, np.float32), 0.0)
    dtabs = {}
    for k, v in tabs0.items():
        tab_shapes[k] = v.shape
        dtabs[k] = nc.dram_tensor(f"tab_{k}", list(v.shape), bf16,
                                  kind="ExternalInput").ap()
    dID8 = nc.dram_tensor("ID8F", [F_ROWS, F_ROWS], f32,
                          kind="ExternalInput").ap()
    dOUT = nc.dram_tensor("OUT", [128, 4 * MCOLS], f32,
                          kind="ExternalOutput").ap()

    with tile.TileContext(nc) as tc:
        with (
            tc.tile_pool(name="consts", bufs=1) as cpool,
            tc.tile_pool(name="xin", bufs=3) as xpool,
            tc.tile_pool(name="work", bufs=2) as wpool,
            tc.tile_pool(name="asb", bufs=2) as aspool,
            tc.tile_pool(name="persist", bufs=1) as ppool,
            tc.tile_pool(name="fin", bufs=4) as fpool,
            tc.tile_pool(name="pga", bufs=1, space="PSUM") as pga,
            tc.tile_pool(name="prc", bufs=1, space="PSUM") as prc,
            tc.tile_pool(name="pas", bufs=1, space="PSUM") as pas,
            tc.tile_pool(name="pt", bufs=1, space="PSUM") as pt,
            tc.tile_pool(name="pg", bufs=1, space="PSUM") as pgp,
        ):
            # ---- constants to SBUF ----
            ct = {}
            for k in tabs0:
                sh = tab_shapes[k]
                ct[k] = cpool.tile(list(sh), bf16, tag=f"c_{k}", name=f"c_{k}")
                nc.sync.dma_start(ct[k][:], dtabs[k][:])
            t0m = cpool.tile([128, MCOLS], f32, tag="t0m", name="t0m")
            nc.sync.dma_start(t0m[:], dT0[:])
            xcs = cpool.tile([128, 4 * MCOLS], f32, tag="xcs", name="xcs")
            nc.sync.dma_start(xcs[:], dXC[:])
            ident8f = cpool.tile([F_ROWS, F_ROWS], f32, tag="id8f", name="ident8f")
            nc.sync.dma_start(ident8f[:], dID8[:])

            Mt = ppool.tile([128, NCHUNK * 32], f32, tag="mega", name="mega")
            xrA = ppool.tile([128, N], bf16, tag="xrA", name="xrA")
            xrB = ppool.tile([128, N], bf16, tag="xrB", name="xrB")
            nc.vector.memset(xrA[:], 0.0)
            nc.vector.memset(xrB[:], 0.0)
            OUTs = ppool.tile([128, 4 * MCOLS], f32, tag="outs", name="outs")

            for c in range(NCHUNK):
                cs = slice(c * N, (c + 1) * N)
                xr = xpool.tile([82, N], bf16, tag="xr", name="xr")
                xi = xpool.tile([82, N], bf16, tag="xi", name="xi")
                nc.sync.dma_start(xr[:], dXR[:, cs])
                nc.sync.dma_start(xi[:], dXI[:, cs])
                xr1 = xpool.tile([41, N], bf16, tag="xr1", name="xr1")
                xi1 = xpool.tile([41, N], bf16, tag="xi1", name="xi1")
                nc.sync.dma_start(xr1[:], dXR[41:82, cs])
                nc.sync.dma_start(xi1[:], dXI[41:82, cs])

                ptr = pt.tile([82, N], f32, tag="tr", name="ptr")
                pti = pt.tile([82, N], f32, tag="ti", name="pti")
                sas = []
                for u, (o, ln) in enumerate(CHUNKS):
                    pXRa = pga.tile([ln, N], f32, tag="xra", name="pXRa")
                    pXIa = pga.tile([ln, N], f32, tag="xia", name="pXIa")
                    pXRc = prc.tile([ln, N], f32, tag="xrc", name="pXRc")
                    pXIc = prc.tile([ln, N], f32, tag="xic", name="pXIc")
                    ga = ct[f"ga_a_{u}"][:]
                    gc = ct[f"ga_c_{u}"][:]
                    xrr = xr[:]
                    xir = xi[:]
                    nc.tensor.matmul(pXRa[:], ga, xrr, start=True, stop=True)
                    nc.tensor.matmul(pXIa[:], ga, xir, start=True, stop=True)
                    nc.tensor.matmul(pXRc[:], gc, xrr, start=True, stop=True)
                    nc.tensor.matmul(pXIc[:], gc, xir, start=True, stop=True)
                    sXRc = wpool.tile([ln, N], f32, tag="sxrc", name="sXRc")
                    sXIc = wpool.tile([ln, N], f32, tag="sxic", name="sXIc")
                    nc.scalar.activation(sXRc[:], pXRc[:], Act.Copy)
                    nc.scalar.activation(sXIc[:], pXIc[:], Act.Copy)
                    p1 = wpool.tile([ln, N], bf16, tag="p1", name="p1")
                    p2 = wpool.tile([ln, N], bf16, tag="p2", name="p2")
                    p3 = wpool.tile([ln, N], bf16, tag="p3", name="p3")
                    p4 = wpool.tile([ln, N], bf16, tag="p4", name="p4")
                    nc.vector.tensor_tensor(p1[:], pXRa[:], sXRc[:], Op.mult)
                    nc.vector.tensor_tensor(p2[:], pXIa[:], sXIc[:], Op.mult)
                    nc.vector.tensor_tensor(p3[:], pXIa[:], sXRc[:], Op.mult)
                    nc.vector.tensor_tensor(p4[:], pXRa[:], sXIc[:], Op.mult)
                    pAs = pas.tile([ln, N], f32, tag="as", name="pAs")
                    w12 = ct[f"w1_T12_{u}"][:]
                    nc.tensor.matmul(pAs[:], w12, p1[:], start=True, stop=False)
                    nc.tensor.matmul(pAs[:], w12, p2[:], start=False, stop=False)
                    nc.tensor.matmul(pAs[:], ct[f"w1_T3_{u}"][:], p3[:],
                                     start=False, stop=False)
                    nc.tensor.matmul(pAs[:], ct[f"w1_T4_{u}"][:], p4[:],
                                     start=False, stop=True)
                    sa = aspool.tile([ln, N], bf16, tag=f"sas{u}", name=f"sa{u}")
                    nc.scalar.activation(sa[:], pAs[:], Act.Copy)
                    sas.append(sa)
                for v, (o, ln) in enumerate(CHUNKS):
                    sar = sas[v][:]
                    nc.tensor.matmul(ptr[:], ct[f"w2r_{v}"][:], sar,
                                     start=(v == 0), stop=(v == 4))
                    nc.tensor.matmul(pti[:], ct[f"w2i_{v}"][:], sar,
                                     start=(v == 0), stop=(v == 4))
                q1 = wpool.tile([82, N], bf16, tag="q1", name="q1")
                q2 = wpool.tile([82, N], bf16, tag="q2", name="q2")
                q3 = wpool.tile([82, N], bf16, tag="q3", name="q3")
                q4 = wpool.tile([82, N], bf16, tag="q4", name="q4")
                nc.vector.tensor_tensor(q1[:], ptr[:], xr[:], Op.mult)
                nc.vector.tensor_tensor(q2[:], pti[:], xi[:], Op.mult)
                nc.vector.tensor_tensor(q3[:], ptr[:], xi[:], Op.mult)
                nc.vector.tensor_tensor(q4[:], pti[:], xr[:], Op.mult)
                s1 = wpool.tile([82, N], bf16, tag="s1", name="s1")
                s2 = wpool.tile([82, N], bf16, tag="s2", name="s2")
                pw = wpool.tile([82, N], bf16, tag="pw", name="pw")
                nc.vector.tensor_tensor(s1[:], xr[:], xr[:], Op.mult)
                nc.vector.tensor_tensor(s2[:], xi[:], xi[:], Op.mult)
                nc.vector.tensor_tensor(pw[:], s1[:], s2[:], Op.add)
                nc.vector.tensor_tensor(xrA[0:41, :], xr[0:41, :], xr1[:], Op.mult)
                nc.vector.tensor_tensor(xrA[64:105, :], xi[0:41, :], xi1[:], Op.mult)
                nc.vector.tensor_tensor(xrB[0:41, :], xi[0:41, :], xr1[:], Op.mult)
                nc.vector.tensor_tensor(xrB[64:105, :], xr[0:41, :], xi1[:], Op.mult)
                pF = prc.tile([F_ROWS, N], f32, tag="xrc", name="pF")
                seq = [("r3_q1", q1), ("r3_q2", q2), ("r3_q3", q3),
                       ("r3_q4", q4), ("r3_pw", pw), ("r3_xrA", xrA),
                       ("r3_xrB", xrB)]
                for si, (wk, rhs) in enumerate(seq):
                    nc.tensor.matmul(pF[:], ct[wk][:], rhs[:],
                                     start=(si == 0), stop=(si == len(seq) - 1))
                sF = wpool.tile([F_ROWS, N], f32, tag="sF", name="sF")
                nc.scalar.activation(sF[:], pF[:], Act.Copy)
                pG = pgp.tile([128, 32], f32, tag="g", name="pG")
                for tq in range(4):
                    nc.tensor.transpose(pG[:, tq * 8:tq * 8 + 8],
                                        sF[:, tq * 128:(tq + 1) * 128],
                                        ident8f[:])
                nc.scalar.activation(Mt[:, c * 32:(c + 1) * 32], pG[:], Act.Copy)

            # ---- final sample-major phase ----
            Mtv = Mt[:].rearrange("p (g k) -> p g k", k=8)

            def msl(k):
                return Mtv[:, :, k]

            hpi = cpool.tile([128, 1], f32, tag="hpi", name="hpi")
            nc.vector.memset(hpi[:], float(np.pi / 2))
            Pht = fpool.tile([128, MCOLS], f32, tag="fA", name="Pht")
            LN10_10 = float(np.log(10.0) / 10.0)
            nc.scalar.activation(Pht[:], t0m[:], Act.Exp, scale=LN10_10)

            def ft(tag="fB"):
                return fpool.tile([128, MCOLS], f32, tag=tag, name="ftmp")

            phi0, phi1 = ft(), ft()
            nc.vector.tensor_tensor(phi0[:], Pht[:], msl(6), Op.mult)
            nc.vector.tensor_tensor(phi1[:], Pht[:], msl(7), Op.mult)
            c0, s0, c1, s1_ = ft("fC"), ft("fC"), ft("fC"), ft("fC")
            nc.scalar.activation(c0[:], phi0[:], Act.Sin, bias=hpi[:])
            nc.scalar.activation(s0[:], phi0[:], Act.Sin)
            nc.scalar.activation(c1[:], phi1[:], Act.Sin, bias=hpi[:])
            nc.scalar.activation(s1_[:], phi1[:], Act.Sin)

            # ix products (all pre-halved through z rows)
            # E_r0 = Pht*(F0 - xcr1*zi - xci1*zr) + xcr0*c0 - xci0*s0
            def xcb(q):
                return xcs[:, q * MCOLS:(q + 1) * MCOLS]

            combos = [
                (0, [(2, 5, -1.0), (3, 4, -1.0)], (0, "c0", +1.0), (1, "s0", -1.0), 0),
                (1, [(2, 4, +1.0), (3, 5, -1.0)], (0, "s0", +1.0), (1, "c0", +1.0), 1),
                (2, [(0, 5, +1.0), (1, 4, -1.0)], (2, "c1", +1.0), (3, "s1", -1.0), 2),
                (3, [(0, 4, +1.0), (1, 5, +1.0)], (2, "s1", +1.0), (3, "c1", +1.0), 3),
            ]
            trig = {"c0": c0, "s0": s0, "c1": c1, "s1": s1_}
            for fidx, prods, term1, term2, outq in combos:
                acc = ft("fD")
                nc.vector.tensor_copy(acc[:], msl(fidx))
                for (ka, kb, sgn) in prods:
                    tmp = ft("fE")
                    nc.vector.tensor_tensor(tmp[:], xcb(ka), msl(kb), Op.mult)
                    nc.vector.tensor_tensor(
                        acc[:], acc[:], tmp[:],
                        Op.add if sgn > 0 else Op.subtract)
                nc.vector.tensor_tensor(acc[:], acc[:], Pht[:], Op.mult)
                for (kc, tkey, sgn) in (term1, term2):
                    tmp = ft("fE")
                    nc.vector.tensor_tensor(tmp[:], xcb(kc), trig[tkey][:], Op.mult)
                    nc.vector.tensor_tensor(
                        acc[:], acc[:], tmp[:],
                        Op.add if sgn > 0 else Op.subtract)
                nc.vector.tensor_copy(
                    OUTs[:, outq * MCOLS:(outq + 1) * MCOLS], acc[:])
            nc.sync.dma_start(dOUT[:], OUTs[:])

    nc.compile()
    return nc


def kernel(**inputs):
    from concourse.bass_utils import run_bass_kernel_spmd

    trace = bool(inputs.pop("_trace", False))
    x_real = np.asarray(inputs["x_real"], dtype=np.float32)
    x_imag = np.asarray(inputs["x_imag"], dtype=np.float32)
    task_info = np.asarray(inputs["task_info"], dtype=np.float32)
    C00 = float(np.asarray(inputs["C00"]).reshape(-1)[0])
    fwm_wr = np.asarray(inputs["fwm_wr"], dtype=np.float32)
    fwm_wi = np.asarray(inputs["fwm_wi"], dtype=np.float32)
    conv1_w = np.asarray(inputs["conv1_w"], dtype=np.float32)
    conv2_w = np.asarray(inputs["conv2_w"], dtype=np.float32)

    B = x_real.shape[0]
    Bc = B // NCORES
    if "nc" not in _CACHED:
        _CACHED["nc"] = _build_program(Bc)
    nc = _CACHED["nc"]

    import ml_dtypes
    bf = ml_dtypes.bfloat16
    tabs = _build_tables(fwm_wr, fwm_wi, conv1_w, conv2_w, C00)
    in_maps = []
    for core in range(NCORES):
        sl = slice(core * Bc, (core + 1) * Bc)
        XR = np.ascontiguousarray(
            x_real[sl].transpose(2, 1, 0).reshape(82, Bc)).astype(bf)
        XI = np.ascontiguousarray(
            x_imag[sl].transpose(2, 1, 0).reshape(82, Bc)).astype(bf)
        t0 = task_info[sl, 0]
        T0M = np.ascontiguousarray(
            t0.reshape(Bc // 512, 4, 128).transpose(2, 0, 1).reshape(128, Bc // 128))
        mcols = Bc // 128
        XC = np.empty((128, 4 * mcols), np.float32)
        for qi, arr in enumerate([x_real[sl, P, 0], x_imag[sl, P, 0],
                                  x_real[sl, P, 1], x_imag[sl, P, 1]]):
            XC[:, qi * mcols:(qi + 1) * mcols] = np.ascontiguousarray(
                arr.reshape(Bc // 512, 4, 128).transpose(2, 0, 1).reshape(128, mcols))
        m = {"XR": XR, "XI": XI, "T0M": T0M, "XC": XC,
             "ID8F": np.eye(F_ROWS, dtype=np.float32)}
        for k, v in tabs.items():
            m[f"tab_{k}"] = v.astype(bf)
        in_maps.append(m)

    res = run_bass_kernel_spmd(nc, in_maps, list(range(NCORES)), trace=trace)
    _CACHED["last_exec_ns"] = res.exec_time_ns

    outs = []
    cols = Bc // 128
    for core in range(NCORES):
        OUT = res.results[core]["OUT"]
        E = np.empty((Bc, 2), np.complex64)
        for q, (dst, im) in enumerate([(0, 0), (0, 1), (1, 0), (1, 1)]):
            O = OUT[:, q * cols:(q + 1) * cols]
            flat = np.ascontiguousarray(
                O.reshape(128, Bc // 512, 4).transpose(1, 2, 0)).reshape(Bc)
            if im == 0:
                E[:, dst] = flat
            else:
                E[:, dst] += 1j * flat.astype(np.complex64)
        outs.append(E)
    return np.concatenate(outs, axis=0)


# revision 12
# speedup vs baseline: 196.6501x; 196.6501x over previous
"""Trainium2 Bass kernel for nn_EqAMPBC (FWM/XPM nonlinear equalizer).

Strategy: pure data-parallel over 8 NeuronCores (batch 131072 -> 16384/core).
Per core, samples are processed in 32 chunks of N=512 in a transposed layout
(features on partitions, samples on the free dim):
  - one-hot fp32r matmuls on TensorE gather the FWM triplet operand rows,
  - VectorE forms the 4 real product tensors per (h, mode),
  - TensorE contracts products -> As -> t (the W-weighted FWM sums),
  - a final TensorE reduction builds 12 per-sample scalars (FWM sums, z,
    phase pre-sums, center taps), which are PE-transposed into a
    sample-major megatile where ScalarE/VectorE apply exp/sin/cos and the
    final complex combine.
All engine work happens on device; the host only reshapes/shards.
"""
import sys
import numpy as np

sys.path.insert(0, "/opt/trn_rl_repo")

M = 41
P = 20
RHO = 1.0
NCORES = 8
N = 512
F_ROWS = 8


def _fwm_index():
    h = M // 2
    ms, ns = [], []
    for m in range(-h, h + 1):
        for n in range(-h, h + 1):
            if m != 0 and n != 0 and abs(m * n) <= RHO * h and abs(m + n) <= h:
                ms.append(m)
                ns.append(n)
    return np.asarray(ms, np.int32), np.asarray(ns, np.int32)


M_IDX, N_IDX = _fwm_index()
HDIM = len(M_IDX)
A_TAP = P + N_IDX
C_TAP = P + M_IDX + N_IDX
D_TAP = P + M_IDX
NPROD = 2 * HDIM
CHUNKS = [(0, 128), (128, 128), (256, 128), (384, 128), (512, 8)]


def _build_tables(fwm_wr, fwm_wi, conv1_w, conv2_w, C00):
    t = {}
    r_all = np.arange(NPROD)
    h_all, j_all = r_all // 2, r_all % 2
    src_a = j_all * 41 + A_TAP[h_all]
    src_c = j_all * 41 + C_TAP[h_all]
    for side, src in (("a", src_a), ("c", src_c)):
        for u, (o, ln) in enumerate(CHUNKS):
            g = np.zeros((82, ln), np.float32)
            g[src[o:o + ln], np.arange(ln)] = 1.0
            t[f"ga_{side}_{u}"] = g
    for u, (o, ln) in enumerate(CHUNKS):
        w_r = np.zeros((ln, ln), np.float32)
        w_i3 = np.zeros((ln, ln), np.float32)
        w_i4 = np.zeros((ln, ln), np.float32)
        loc = np.arange(ln)
        w_r[loc, 2 * (loc // 2)] = 1.0
        w_i3[loc, 2 * (loc // 2) + 1] = 1.0
        w_i4[loc, 2 * (loc // 2) + 1] = -1.0
        t[f"w1_T12_{u}"] = w_r
        t[f"w1_T3_{u}"] = w_i3
        t[f"w1_T4_{u}"] = w_i4
    for v, (o, ln) in enumerate(CHUNKS):
        wr = np.zeros((ln, 82), np.float32)
        wi = np.zeros((ln, 82), np.float32)
        for rl in range(ln):
            h = (o + rl) // 2
            part = rl % 2
            for i in range(2):
                col = i * 41 + D_TAP[h]
                if part == 0:
                    wr[rl, col] += fwm_wr[i, h]
                    wi[rl, col] += fwm_wi[i, h]
                else:
                    wr[rl, col] += -fwm_wi[i, h]
                    wi[rl, col] += fwm_wr[i, h]
        t[f"w2r_{v}"] = wr
        t[f"w2i_{v}"] = wi
    w1z = conv1_w.copy(); w1z[P] = 0.0
    w2z = conv2_w.copy(); w2z[P] = 0.0
    q1 = np.zeros((82, F_ROWS), np.float32)
    q2 = np.zeros((82, F_ROWS), np.float32)
    q3 = np.zeros((82, F_ROWS), np.float32)
    q4 = np.zeros((82, F_ROWS), np.float32)
    for i in range(2):
        rows = np.arange(41) + i * 41
        q1[rows, 2 * i + 0] = 0.5
        q2[rows, 2 * i + 0] = -0.5
        q3[rows, 2 * i + 1] = 0.5
        q4[rows, 2 * i + 1] = 0.5
    t["r3_q1"], t["r3_q2"], t["r3_q3"], t["r3_q4"] = q1, q2, q3, q4
    pw = np.zeros((82, F_ROWS), np.float32)
    for i in range(2):
        for tap in range(41):
            r = i * 41 + tap
            pw[r, 6] += (2.0 if i == 0 else 1.0) * w1z[tap]
            pw[r, 7] += (2.0 if i == 1 else 1.0) * w1z[tap]
        pw[i * 41 + P, 6] += 0.5 * C00
        pw[i * 41 + P, 7] += 0.5 * C00
    t["r3_pw"] = pw
    xrA = np.zeros((128, F_ROWS), np.float32)
    xrA[np.arange(41), 4] = 0.5 * w2z
    xrA[np.arange(41) + 64, 4] = 0.5 * w2z
    xrB = np.zeros((128, F_ROWS), np.float32)
    xrB[np.arange(41), 5] = 0.5 * w2z
    xrB[np.arange(41) + 64, 5] = -0.5 * w2z
    t["r3_xrA"], t["r3_xrB"] = xrA, xrB
    t["ident8"] = np.eye(F_ROWS, dtype=np.float32)
    return t


_CACHED = {}


def _build_program(Bc):
    import concourse.bacc as bacc
    import concourse.mybir as mybir
    import concourse.tile as tile

    f32 = mybir.dt.float32
    bf16 = mybir.dt.bfloat16
    Act = mybir.ActivationFunctionType
    Op = mybir.AluOpType
    NCHUNK = Bc // N
    MCOLS = Bc // 128          # megatile cols per quantity-slot group

    nc = bacc.Bacc("TRN2", target_bir_lowering=False, debug=False,
                   num_devices=NCORES)

    dXR = nc.dram_tensor("XR", [82, Bc], bf16, kind="ExternalInput").ap()
    dXI = nc.dram_tensor("XI", [82, Bc], bf16, kind="ExternalInput").ap()
    dXC = nc.dram_tensor("XC", [128, 4 * MCOLS], f32, kind="ExternalInput").ap()
    dT0 = nc.dram_tensor("T0M", [128, MCOLS], f32, kind="ExternalInput").ap()
    tab_shapes = {}
    tabs0 = _build_tables(np.zeros((2, HDIM), np.float32),
                          np.zeros((2, HDIM), np.float32),
                          np.zeros(M, np.float32), np.zeros(M, np.float32), 0.0)
    dtabs = {}
    for k, v in tabs0.items():
        tab_shapes[k] = v.shape
        dtabs[k] = nc.dram_tensor(f"tab_{k}", list(v.shape), bf16,
                                  kind="ExternalInput").ap()
    dID8 = nc.dram_tensor("ID8F", [F_ROWS, F_ROWS], f32,
                          kind="ExternalInput").ap()
    dOUT = nc.dram_tensor("OUT", [128, 4 * MCOLS], f32,
                          kind="ExternalOutput").ap()

    with tile.TileContext(nc) as tc:
        with (
            tc.tile_pool(name="consts", bufs=1) as cpool,
            tc.tile_pool(name="xin", bufs=3) as xpool,
            tc.tile_pool(name="work", bufs=2) as wpool,
            tc.tile_pool(name="asb", bufs=2) as aspool,
            tc.tile_pool(name="persist", bufs=1) as ppool,
            tc.tile_pool(name="fin", bufs=4) as fpool,
            tc.tile_pool(name="pga", bufs=1, space="PSUM") as pga,
            tc.tile_pool(name="prc", bufs=1, space="PSUM") as prc,
            tc.tile_pool(name="pas", bufs=1, space="PSUM") as pas,
            tc.tile_pool(name="pt", bufs=1, space="PSUM") as pt,
            tc.tile_pool(name="pg", bufs=1, space="PSUM") as pgp,
        ):
            # ---- constants to SBUF ----
            ct = {}
            for k in tabs0:
                sh = tab_shapes[k]
                ct[k] = cpool.tile(list(sh), bf16, tag=f"c_{k}", name=f"c_{k}")
                nc.sync.dma_start(ct[k][:], dtabs[k][:])
            t0m = cpool.tile([128, MCOLS], f32, tag="t0m", name="t0m")
            nc.sync.dma_start(t0m[:], dT0[:])
            xcs = cpool.tile([128, 4 * MCOLS], f32, tag="xcs", name="xcs")
            nc.sync.dma_start(xcs[:], dXC[:])
            ident8f = cpool.tile([F_ROWS, F_ROWS], f32, tag="id8f", name="ident8f")
            nc.sync.dma_start(ident8f[:], dID8[:])

            Mt = ppool.tile([128, NCHUNK * 32], f32, tag="mega", name="mega")
            xrA = ppool.tile([128, N], bf16, tag="xrA", name="xrA")
            xrB = ppool.tile([128, N], bf16, tag="xrB", name="xrB")
            nc.vector.memset(xrA[:], 0.0)
            nc.vector.memset(xrB[:], 0.0)
            OUTs = ppool.tile([128, 4 * MCOLS], f32, tag="outs", name="outs")

            for c in range(NCHUNK):
                cs = slice(c * N, (c + 1) * N)
                xr = xpool.tile([82, N], bf16, tag="xr", name="xr")
                xi = xpool.tile([82, N], bf16, tag="xi", name="xi")
                nc.sync.dma_start(xr[:], dXR[:, cs])
                nc.sync.dma_start(xi[:], dXI[:, cs])
                xr1 = xpool.tile([41, N], bf16, tag="xr1", name="xr1")
                xi1 = xpool.tile([41, N], bf16, tag="xi1", name="xi1")
                nc.sync.dma_start(xr1[:], dXR[41:82, cs])
                nc.sync.dma_start(xi1[:], dXI[41:82, cs])

                ptr = pt.tile([82, N], f32, tag="tr", name="ptr")
                pti = pt.tile([82, N], f32, tag="ti", name="pti")
                sas = []
                for u, (o, ln) in enumerate(CHUNKS):
                    pXRa = pga.tile([ln, N], f32, tag="xra", name="pXRa")
                    pXIa = pga.tile([ln, N], f32, tag="xia", name="pXIa")
                    pXRc = prc.tile([ln, N], f32, tag="xrc", name="pXRc")
                    pXIc = prc.tile([ln, N], f32, tag="xic", name="pXIc")
                    ga = ct[f"ga_a_{u}"][:]
                    gc = ct[f"ga_c_{u}"][:]
                    xrr = xr[:]
                    xir = xi[:]
                    nc.tensor.matmul(pXRa[:], ga, xrr, start=True, stop=True)
                    nc.tensor.matmul(pXIa[:], ga, xir, start=True, stop=True)
                    nc.tensor.matmul(pXRc[:], gc, xrr, start=True, stop=True)
                    nc.tensor.matmul(pXIc[:], gc, xir, start=True, stop=True)
                    sXRc = wpool.tile([ln, N], bf16, tag="sxrc", name="sXRc")
                    sXIc = wpool.tile([ln, N], bf16, tag="sxic", name="sXIc")
                    nc.scalar.activation(sXRc[:], pXRc[:], Act.Copy)
                    nc.scalar.activation(sXIc[:], pXIc[:], Act.Copy)

                    p1 = wpool.tile([ln, N], bf16, tag="p1", name="p1")
                    p2 = wpool.tile([ln, N], bf16, tag="p2", name="p2")
                    p3 = wpool.tile([ln, N], bf16, tag="p3", name="p3")
                    p4 = wpool.tile([ln, N], bf16, tag="p4", name="p4")
                    sXRa = wpool.tile([ln, N], bf16, tag="sxra", name="sXRa")
                    sXIa = wpool.tile([ln, N], bf16, tag="sxia", name="sXIa")
                    nc.vector.tensor_copy(sXRa[:], pXRa[:])
                    nc.vector.tensor_copy(sXIa[:], pXIa[:])
                    nc.vector.tensor_tensor(p1[:], sXRa[:], sXRc[:], Op.mult)
                    nc.vector.tensor_tensor(p2[:], sXIa[:], sXIc[:], Op.mult)
                    nc.vector.tensor_tensor(p3[:], sXIa[:], sXRc[:], Op.mult)
                    nc.vector.tensor_tensor(p4[:], sXRa[:], sXIc[:], Op.mult)
                    pAs = pas.tile([ln, N], f32, tag="as", name="pAs")
                    w12 = ct[f"w1_T12_{u}"][:]
                    nc.tensor.matmul(pAs[:], w12, p1[:], start=True, stop=False)
                    nc.tensor.matmul(pAs[:], w12, p2[:], start=False, stop=False)
                    nc.tensor.matmul(pAs[:], ct[f"w1_T3_{u}"][:], p3[:],
                                     start=False, stop=False)
                    nc.tensor.matmul(pAs[:], ct[f"w1_T4_{u}"][:], p4[:],
                                     start=False, stop=True)
                    sa = aspool.tile([ln, N], bf16, tag=f"sas{u}", name=f"sa{u}")
                    nc.scalar.activation(sa[:], pAs[:], Act.Copy)
                    sas.append(sa)
                for v, (o, ln) in enumerate(CHUNKS):
                    sar = sas[v][:]
                    nc.tensor.matmul(ptr[:], ct[f"w2r_{v}"][:], sar,
                                     start=(v == 0), stop=(v == 4))
                    nc.tensor.matmul(pti[:], ct[f"w2i_{v}"][:], sar,
                                     start=(v == 0), stop=(v == 4))
                str_ = wpool.tile([82, N], bf16, tag="str", name="str_")
                sti = wpool.tile([82, N], bf16, tag="sti", name="sti")
                nc.scalar.activation(str_[:], ptr[:], Act.Copy)
                nc.scalar.activation(sti[:], pti[:], Act.Copy)
                q1 = wpool.tile([82, N], bf16, tag="q1", name="q1")
                q2 = wpool.tile([82, N], bf16, tag="q2", name="q2")
                q3 = wpool.tile([82, N], bf16, tag="q3", name="q3")
                q4 = wpool.tile([82, N], bf16, tag="q4", name="q4")
                nc.vector.tensor_tensor(q1[:], str_[:], xr[:], Op.mult)
                nc.vector.tensor_tensor(q2[:], sti[:], xi[:], Op.mult)
                nc.vector.tensor_tensor(q3[:], str_[:], xi[:], Op.mult)
                nc.vector.tensor_tensor(q4[:], sti[:], xr[:], Op.mult)
                s1 = wpool.tile([82, N], bf16, tag="s1", name="s1")
                s2 = wpool.tile([82, N], bf16, tag="s2", name="s2")
                pw = wpool.tile([82, N], bf16, tag="pw", name="pw")
                nc.vector.tensor_tensor(s1[:], xr[:], xr[:], Op.mult)
                nc.vector.tensor_tensor(s2[:], xi[:], xi[:], Op.mult)
                nc.vector.tensor_tensor(pw[:], s1[:], s2[:], Op.add)
                nc.vector.tensor_tensor(xrA[0:41, :], xr[0:41, :], xr1[:], Op.mult)
                nc.vector.tensor_tensor(xrA[64:105, :], xi[0:41, :], xi1[:], Op.mult)
                nc.vector.tensor_tensor(xrB[0:41, :], xi[0:41, :], xr1[:], Op.mult)
                nc.vector.tensor_tensor(xrB[64:105, :], xr[0:41, :], xi1[:], Op.mult)
                pF = prc.tile([F_ROWS, N], f32, tag="xrc", name="pF")
                seq = [("r3_q1", q1), ("r3_q2", q2), ("r3_q3", q3),
                       ("r3_q4", q4), ("r3_pw", pw), ("r3_xrA", xrA),
                       ("r3_xrB", xrB)]
                for si, (wk, rhs) in enumerate(seq):
                    nc.tensor.matmul(pF[:], ct[wk][:], rhs[:],
                                     start=(si == 0), stop=(si == len(seq) - 1))
                sF = wpool.tile([F_ROWS, N], f32, tag="sF", name="sF")
                nc.scalar.activation(sF[:], pF[:], Act.Copy)
                pG = pgp.tile([128, 32], f32, tag="g", name="pG")
                for tq in range(4):
                    nc.tensor.transpose(pG[:, tq * 8:tq * 8 + 8],
                                        sF[:, tq * 128:(tq + 1) * 128],
                                        ident8f[:])
                nc.scalar.activation(Mt[:, c * 32:(c + 1) * 32], pG[:], Act.Copy)

            # ---- final sample-major phase ----
            Mtv = Mt[:].rearrange("p (g k) -> p g k", k=8)

            def msl(k):
                return Mtv[:, :, k]

            hpi = cpool.tile([128, 1], f32, tag="hpi", name="hpi")
            nc.vector.memset(hpi[:], float(np.pi / 2))
            Pht = fpool.tile([128, MCOLS], f32, tag="fA", name="Pht")
            LN10_10 = float(np.log(10.0) / 10.0)
            nc.scalar.activation(Pht[:], t0m[:], Act.Exp, scale=LN10_10)

            def ft(tag="fB"):
                return fpool.tile([128, MCOLS], f32, tag=tag, name="ftmp")

            phi0, phi1 = ft(), ft()
            nc.vector.tensor_tensor(phi0[:], Pht[:], msl(6), Op.mult)
            nc.vector.tensor_tensor(phi1[:], Pht[:], msl(7), Op.mult)
            c0, s0, c1, s1_ = ft("fC"), ft("fC"), ft("fC"), ft("fC")
            nc.scalar.activation(c0[:], phi0[:], Act.Sin, bias=hpi[:])
            nc.scalar.activation(s0[:], phi0[:], Act.Sin)
            nc.scalar.activation(c1[:], phi1[:], Act.Sin, bias=hpi[:])
            nc.scalar.activation(s1_[:], phi1[:], Act.Sin)

            # ix products (all pre-halved through z rows)
            # E_r0 = Pht*(F0 - xcr1*zi - xci1*zr) + xcr0*c0 - xci0*s0
            def xcb(q):
                return xcs[:, q * MCOLS:(q + 1) * MCOLS]

            combos = [
                (0, [(2, 5, -1.0), (3, 4, -1.0)], (0, "c0", +1.0), (1, "s0", -1.0), 0),
                (1, [(2, 4, +1.0), (3, 5, -1.0)], (0, "s0", +1.0), (1, "c0", +1.0), 1),
                (2, [(0, 5, +1.0), (1, 4, -1.0)], (2, "c1", +1.0), (3, "s1", -1.0), 2),
                (3, [(0, 4, +1.0), (1, 5, +1.0)], (2, "s1", +1.0), (3, "c1", +1.0), 3),
            ]
            trig = {"c0": c0, "s0": s0, "c1": c1, "s1": s1_}
            for fidx, prods, term1, term2, outq in combos:
                acc = ft("fD")
                nc.vector.tensor_copy(acc[:], msl(fidx))
                for (ka, kb, sgn) in prods:
                    tmp = ft("fE")
                    nc.vector.tensor_tensor(tmp[:], xcb(ka), msl(kb), Op.mult)
                    nc.vector.tensor_tensor(
                        acc[:], acc[:], tmp[:],
                        Op.add if sgn > 0 else Op.subtract)
                nc.vector.tensor_tensor(acc[:], acc[:], Pht[:], Op.mult)
                for (kc, tkey, sgn) in (term1, term2):
                    tmp = ft("fE")
                    nc.vector.tensor_tensor(tmp[:], xcb(kc), trig[tkey][:], Op.mult)
                    nc.vector.tensor_tensor(
                        acc[:], acc[:], tmp[:],
                        Op.add if sgn > 0 else Op.subtract)
                nc.vector.tensor_copy(
                    OUTs[:, outq * MCOLS:(outq + 1) * MCOLS], acc[:])
            nc.sync.dma_start(dOUT[:], OUTs[:])

    nc.compile()
    return nc


def kernel(**inputs):
    from concourse.bass_utils import run_bass_kernel_spmd

    trace = bool(inputs.pop("_trace", False))
    x_real = np.asarray(inputs["x_real"], dtype=np.float32)
    x_imag = np.asarray(inputs["x_imag"], dtype=np.float32)
    task_info = np.asarray(inputs["task_info"], dtype=np.float32)
    C00 = float(np.asarray(inputs["C00"]).reshape(-1)[0])
    fwm_wr = np.asarray(inputs["fwm_wr"], dtype=np.float32)
    fwm_wi = np.asarray(inputs["fwm_wi"], dtype=np.float32)
    conv1_w = np.asarray(inputs["conv1_w"], dtype=np.float32)
    conv2_w = np.asarray(inputs["conv2_w"], dtype=np.float32)

    B = x_real.shape[0]
    Bc = B // NCORES
    if "nc" not in _CACHED:
        _CACHED["nc"] = _build_program(Bc)
    nc = _CACHED["nc"]

    import ml_dtypes
    bf = ml_dtypes.bfloat16
    tabs = _build_tables(fwm_wr, fwm_wi, conv1_w, conv2_w, C00)
    in_maps = []
    for core in range(NCORES):
        sl = slice(core * Bc, (core + 1) * Bc)
        XR = np.ascontiguousarray(
            x_real[sl].transpose(2, 1, 0).reshape(82, Bc)).astype(bf)
        XI = np.ascontiguousarray(
            x_imag[sl].transpose(2, 1, 0).reshape(82, Bc)).astype(bf)
        t0 = task_info[sl, 0]
        T0M = np.ascontiguousarray(
            t0.reshape(Bc // 512, 4, 128).transpose(2, 0, 1).reshape(128, Bc // 128))
        mcols = Bc // 128
        XC = np.empty((128, 4 * mcols), np.float32)
        for qi, arr in enumerate([x_real[sl, P, 0], x_imag[sl, P, 0],
                                  x_real[sl, P, 1], x_imag[sl, P, 1]]):
            XC[:, qi * mcols:(qi + 1) * mcols] = np.ascontiguousarray(
                arr.reshape(Bc // 512, 4, 128).transpose(2, 0, 1).reshape(128, mcols))
        m = {"XR": XR, "XI": XI, "T0M": T0M, "XC": XC,
             "ID8F": np.eye(F_ROWS, dtype=np.float32)}
        for k, v in tabs.items():
            m[f"tab_{k}"] = v.astype(bf)
        in_maps.append(m)

    res = run_bass_kernel_spmd(nc, in_maps, list(range(NCORES)), trace=trace)
    _CACHED["last_exec_ns"] = res.exec_time_ns

    outs = []
    cols = Bc // 128
    for core in range(NCORES):
        OUT = res.results[core]["OUT"]
        E = np.empty((Bc, 2), np.complex64)
        for q, (dst, im) in enumerate([(0, 0), (0, 1), (1, 0), (1, 1)]):
            O = OUT[:, q * cols:(q + 1) * cols]
            flat = np.ascontiguousarray(
                O.reshape(128, Bc // 512, 4).transpose(1, 2, 0)).reshape(Bc)
            if im == 0:
                E[:, dst] = flat
            else:
                E[:, dst] += 1j * flat.astype(np.complex64)
        outs.append(E)
    return np.concatenate(outs, axis=0)
